# revision 8
# baseline (speedup 1.0000x reference)
"""Trainium2 Bass kernel for nn_CoordinatesFusion.

Reference computation (per batch element b, T=2048, D=512, DH=1536):
    left_out  = gelu(left_embed  @ Wl + bl)            [T, D]
    right_out = gelu(right_embed @ Wr + br)            [T, D]
    body_out  = gelu(body_embed  @ Wb + bb)            [T, D]
    attn = softmax(right_out @ left_out^T, axis=-1)    [T, T]
    fuse = attn @ body_out                             [T, D]
    fuse = LN(fuse @ Wo + bo; ln_g, ln_b)
    h = gelu(fuse @ ir_W1 + ir_b1) + fuse
    h = LN(h; ir_ln_g, ir_ln_b)
    h = gelu(h @ ir_W2 + ir_b2)                        [T, DH]
    out = h @ ir_W3 + ir_b3                            [T, D]

Sharding: data-parallel over batch B=8 across the 8 NeuronCores (core c
handles batch element c); the small linear/LayerNorm params are replicated.

Layout strategy per core: activations that feed a matmul's contraction over
features are kept feature-major ("transposed", [D, T] with features on
partitions); activations contracted over tokens are token-major. The three
embeddings are transposed once on the PE (fp32-exact); all large matmuls run
as float32r (fp32 data, single-pass PE mode: full speed at moving dim >= 256).
"""

import os
from contextlib import ExitStack

import numpy as np

import concourse.bacc as bacc
import concourse.bass as bass
import concourse.mybir as mybir
import concourse.tile as tile
from concourse.bass_utils import run_bass_kernel_spmd
from concourse.masks import make_identity

P = 128
D = 512
DH = 1536
KD = D // P          # 4 feature sub-tiles of 128
NM = DH // P         # 12 hidden sub-tiles of 128
F32 = mybir.dt.float32
F32R = mybir.dt.float32r
EPS = 1e-5
AF = mybir.ActivationFunctionType
OP = mybir.AluOpType

N_CORES = 8
T_FULL = 2048


def _mm(ap, dt):
    """Bitcast a matmul-operand AP to the requested PE dtype."""
    if ap.dtype == dt:
        return ap
    return ap.bitcast(dt)


def build(T=T_FULL, n_cores=N_CORES, mm_dt=F32R, s_dt=F32R, trace_sim=False):
    """Build (and bacc-compile) the single-core SPMD Bass module."""
    NT = T // P                      # token tiles (16)
    CH = min(512, T)                 # moving-dim chunk
    NCH = T // CH                    # chunks over tokens (4)

    nc = bacc.Bacc(
        "TRN2", target_bir_lowering=False, debug=False, num_devices=n_cores
    )

    dr = {}
    for name in ("left_embed", "right_embed", "body_embed"):
        dr[name] = nc.dram_tensor(name, [T, D], F32, kind="ExternalInput").ap()
    for name in ("Wl", "Wr", "Wb", "Wo", "ir_W1"):
        dr[name] = nc.dram_tensor(name, [D, D], F32, kind="ExternalInput").ap()
    dr["ir_W2"] = nc.dram_tensor("ir_W2", [D, DH], F32, kind="ExternalInput").ap()
    dr["ir_W3"] = nc.dram_tensor("ir_W3", [DH, D], F32, kind="ExternalInput").ap()
    for name in ("bl", "br", "bb", "bo", "ln_g", "ln_b", "ir_b1",
                 "ir_ln_g", "ir_ln_b", "ir_b3"):
        dr[name] = nc.dram_tensor(name, [D], F32, kind="ExternalInput").ap()
    dr["ir_b2"] = nc.dram_tensor("ir_b2", [DH], F32, kind="ExternalInput").ap()
    out_dram = nc.dram_tensor("out", [T, D], F32, kind="ExternalOutput").ap()

    with tile.TileContext(nc, trace_sim=trace_sim) as tc:
        _body(tc, dr, out_dram, T, NT, CH, NCH, mm_dt, s_dt)

    nc.compile()
    return nc


def _body(tc, dr, out_dram, T, NT, CH, NCH, mm_dt, s_dt):
    nc = tc.nc
    with ExitStack() as octx:
        # long-lived pools
        consts = octx.enter_context(tc.tile_pool(name="consts", bufs=1))
        # released manually after phase B so phase C can use its space
        pR = tc.alloc_tile_pool(name="persistR", bufs=1, side="right")
        dram = octx.enter_context(tc.tile_pool(name="dram", bufs=1, space="DRAM"))
        psb = octx.enter_context(tc.tile_pool(name="psb", bufs=5, space="PSUM"))
        pst = octx.enter_context(tc.tile_pool(name="pst", bufs=3, space="PSUM"))

        # ---- constants -------------------------------------------------
        ident = consts.tile([P, P], F32, tag="ident")
        make_identity(nc, ident)
        eps_t = consts.tile([P, 1], F32, tag="eps")
        nc.vector.memset(eps_t, EPS)

        def load_w(pool, name, cols, tag):
            t = pool.tile([P, KD if name != "ir_W3" else NM, cols], F32R, tag=tag)
            t_ = dr[name].rearrange("(ko p) n -> p ko n", p=P).bitcast(F32R)
            nc.sync.dma_start(t, t_)
            return t

        def load_bias_part(pool, name, n, tag):
            # per-partition bias layout [P, n]: element (p, j) = vec[j*P + p]
            t = pool.tile([P, n], F32, tag=tag)
            nc.sync.dma_start(t, dr[name].rearrange("(ko p) -> p ko", p=P))
            return t

        def load_bcast(pool, name, tag):
            # broadcast a [n]-vector across all 128 partitions -> [P, n]
            v = dr[name]
            n = v.shape[0]
            t = pool.tile([P, n], F32, tag=tag)
            src = bass.AP(tensor=v.tensor, offset=v.offset, ap=[[0, P], *v.ap])
            nc.gpsimd.dma_start(out=t, in_=src)
            return t

        bo_bc = load_bcast(consts, "bo", "bo")
        b1_bc = load_bcast(consts, "ir_b1", "b1")
        b2_sb = load_bias_part(consts, "ir_b2", NM, "b2")
        b3_bc = load_bcast(consts, "ir_b3", "b3")
        lng_bc = load_bcast(consts, "ln_g", "lng")
        lnb_bc = load_bcast(consts, "ln_b", "lnb")
        ilng_bc = load_bcast(consts, "ir_ln_g", "ilng")
        ilnb_bc = load_bcast(consts, "ir_ln_b", "ilnb")

        # persistent (A..B) activations, right heap side
        left_T = pR.tile([P, KD, T], F32R, tag="leftT")
        body_nat = pR.tile([P, NT, D], F32R, tag="bodyN")
        # right projection spilled to DRAM (written once in A, read per q-tile)
        rtd = dram.tile([KD, P, T], F32R, tag="rtd")

        # ---- phase A: transpose embeddings + L1 projections ------------
        with ExitStack() as actx:
            wA = actx.enter_context(tc.tile_pool(name="wA", bufs=1))
            embp = actx.enter_context(tc.tile_pool(name="embp", bufs=1))
            natp = actx.enter_context(tc.tile_pool(name="natp", bufs=3))

            Wl_sb = load_w(wA, "Wl", D, "Wl")
            Wr_sb = load_w(wA, "Wr", D, "Wr")
            Wb_sb = load_w(wA, "Wb", D, "Wb")
            bl_sb = load_bias_part(wA, "bl", KD, "bl")
            br_sb = load_bias_part(wA, "br", KD, "br")
            bb_bc = load_bcast(wA, "bb", "bb")

            def transpose_in(emb):
                embT = embp.tile([P, KD, T], F32R, tag="embT")
                for i in range(NT):
                    nat = natp.tile([P, D], F32, tag="nat")
                    nc.sync.dma_start(nat, emb[i * P:(i + 1) * P, :])
                    for j in range(KD):
                        ps = pst.tile([P, P], F32, tag="ptr")
                        nc.tensor.transpose(ps, nat[:, j * P:(j + 1) * P], ident)
                        nc.vector.tensor_copy(
                            out=embT[:, j, i * P:(i + 1) * P], in_=ps)
                return embT

            # left: output feature-major into resident left_T
            embT = transpose_in(dr["left_embed"])
            for m in range(KD):
                for c in range(NCH):
                    ps = psb.tile([P, CH], F32, tag="pmm")
                    for k in range(KD):
                        nc.tensor.matmul(
                            ps,
                            _mm(Wl_sb[:, k, m * P:(m + 1) * P], mm_dt),
                            _mm(embT[:, k, c * CH:(c + 1) * CH], mm_dt),
                            start=(k == 0), stop=(k == KD - 1),
                        )
                    nc.scalar.activation(
                        out=left_T[:, m, c * CH:(c + 1) * CH], in_=ps,
                        func=AF.Gelu, bias=bl_sb[:, m:m + 1], scale=1.0,
                    )

            # right: feature-major, spilled to DRAM
            embT = transpose_in(dr["right_embed"])
            for m in range(KD):
                for c in range(NCH):
                    ps = psb.tile([P, CH], F32, tag="pmm")
                    for k in range(KD):
                        nc.tensor.matmul(
                            ps,
                            _mm(Wr_sb[:, k, m * P:(m + 1) * P], mm_dt),
                            _mm(embT[:, k, c * CH:(c + 1) * CH], mm_dt),
                            start=(k == 0), stop=(k == KD - 1),
                        )
                    rp = natp.tile([P, CH], F32R, tag="rproj")
                    nc.scalar.activation(
                        out=rp, in_=ps,
                        func=AF.Gelu, bias=br_sb[:, m:m + 1], scale=1.0,
                    )
                    nc.sync.dma_start(rtd[m, :, c * CH:(c + 1) * CH], rp)

            # body: token-major into resident body_nat
            embT = transpose_in(dr["body_embed"])
            for i in range(NT):
                ps = psb.tile([P, D], F32, tag="pmm")
                for k in range(KD):
                    nc.tensor.matmul(
                        ps,
                        _mm(embT[:, k, i * P:(i + 1) * P], mm_dt),
                        _mm(Wb_sb[:, k, :], mm_dt),
                        start=(k == 0), stop=(k == KD - 1),
                    )
                nc.vector.tensor_add(out=ps, in0=ps, in1=bb_bc)
                nc.scalar.activation(out=body_nat[:, i, :], in_=ps, func=AF.Gelu)

        # ---- phase B: attention, per 128-query tile --------------------
        pZ = octx.enter_context(tc.tile_pool(name="pZ", bufs=1))
        # z_sb accumulates fuse @ Wo + bo (pre-LN), token-major
        z_sb = pZ.tile([P, NT, D], F32, tag="zbuf")

        bctx = ExitStack()
        attn = bctx.enter_context(tc.tile_pool(name="attn", bufs=2, side="right"))
        wB = bctx.enter_context(tc.tile_pool(name="wB", bufs=1))
        midp = bctx.enter_context(tc.tile_pool(name="midp", bufs=3))
        small = bctx.enter_context(tc.tile_pool(name="small", bufs=4))

        Wo_sb = load_w(wB, "Wo", D, "Wo")

        for i in range(NT):
            rt = midp.tile([P, KD, P], F32R, tag="rt")
            nc.sync.dma_start(
                rt, rtd[:, :, i * P:(i + 1) * P].rearrange("k p t -> p k t"))
            s_banks = []
            for c in range(NCH):
                ps = psb.tile([P, CH], F32, tag="pmm")
                for k in range(KD):
                    nc.tensor.matmul(
                        ps,
                        _mm(rt[:, k, :], s_dt),
                        _mm(left_T[:, k, c * CH:(c + 1) * CH], s_dt),
                        start=(k == 0), stop=(k == KD - 1),
                    )
                s_banks.append(ps)

            mx = small.tile([P, NCH], F32, tag="mx")
            for c, ps in enumerate(s_banks):
                nc.vector.reduce_max(out=mx[:, c:c + 1], in_=ps,
                                     axis=mybir.AxisListType.X)
            m1 = small.tile([P, 1], F32, tag="m1")
            nc.vector.reduce_max(out=m1, in_=mx, axis=mybir.AxisListType.X)
            negm = small.tile([P, 1], F32, tag="negm")
            nc.vector.tensor_scalar_mul(negm, m1, -1.0)

            P_sb = attn.tile([P, T], F32, tag="Psb")
            sums = small.tile([P, NCH], F32, tag="sums")
            for c, ps in enumerate(s_banks):
                nc.scalar.activation(
                    out=P_sb[:, c * CH:(c + 1) * CH], in_=ps, func=AF.Exp,
                    bias=negm, scale=1.0, accum_out=sums[:, c:c + 1],
                )
            s1 = small.tile([P, 1], F32, tag="s1")
            nc.vector.reduce_sum(out=s1, in_=sums, axis=mybir.AxisListType.X)
            rs = small.tile([P, 1], F32, tag="rs")
            nc.vector.reciprocal(rs, s1)

            PT = attn.tile([P, NT, P], F32R, tag="PT")
            for k in range(NT):
                ps = pst.tile([P, P], F32, tag="ptr")
                nc.tensor.transpose(ps, P_sb[:, k * P:(k + 1) * P], ident)
                nc.vector.tensor_copy(out=PT[:, k, :], in_=ps)

            pv = psb.tile([P, D], F32, tag="pmm")
            for k in range(NT):
                nc.tensor.matmul(
                    pv,
                    _mm(PT[:, k, :], mm_dt),
                    _mm(body_nat[:, k, :], mm_dt),
                    start=(k == 0), stop=(k == NT - 1),
                )
            fuse = midp.tile([P, D], F32, tag="fuse")
            nc.vector.tensor_scalar_mul(fuse, pv, rs)  # normalize by row sum

            fT = midp.tile([P, KD, P], F32R, tag="fT")
            for j in range(KD):
                ps = pst.tile([P, P], F32, tag="ptr")
                nc.tensor.transpose(ps, fuse[:, j * P:(j + 1) * P], ident)
                nc.vector.tensor_copy(out=fT[:, j, :], in_=ps)

            zp = psb.tile([P, D], F32, tag="pmm")
            for k in range(KD):
                nc.tensor.matmul(
                    zp,
                    _mm(fT[:, k, :], mm_dt),
                    _mm(Wo_sb[:, k, :], mm_dt),
                    start=(k == 0), stop=(k == KD - 1),
                )
            nc.vector.tensor_add(out=z_sb[:, i, :], in0=zp, in1=bo_bc)

        bctx.close()  # release attention pools
        pR.release()  # left_T / body_nat no longer needed

        # ---- phase C: LN -> MLP ---------------------------------------
        cctx = ExitStack()
        wC = cctx.enter_context(tc.tile_pool(name="wC", bufs=1))
        xTp = cctx.enter_context(tc.tile_pool(name="xTp", bufs=1))
        h3p = cctx.enter_context(tc.tile_pool(name="h3p", bufs=1))
        midp = cctx.enter_context(tc.tile_pool(name="midpC", bufs=3))
        small = cctx.enter_context(tc.tile_pool(name="smallC", bufs=4))

        W1_sb = load_w(wC, "ir_W1", D, "W1")
        W2_sb = load_w(wC, "ir_W2", DH, "W2")
        W3_sb = load_w(wC, "ir_W3", D, "W3")

        def layernorm_batch(buf, g_bc, b_bc):
            # buf: [P, NT, D] token-major; normalize each row over D
            mv = small.tile([P, NT, 2], F32, tag="mv")
            for i in range(NT):
                st = small.tile([P, 6], F32, tag="st")
                nc.vector.bn_stats(out=st, in_=buf[:, i, :])
                nc.vector.bn_aggr(out=mv[:, i, :], in_=st)
            sd = small.tile([P, NT], F32, tag="sd")
            nc.scalar.activation(out=sd, in_=mv[:, :, 1:2], func=AF.Sqrt,
                                 bias=eps_t, scale=1.0)
            rstd = small.tile([P, NT], F32, tag="rstd")
            nc.vector.reciprocal(rstd, sd)
            for i in range(NT):
                nc.vector.tensor_scalar(
                    out=buf[:, i, :], in0=buf[:, i, :],
                    scalar1=mv[:, i, 0:1], scalar2=rstd[:, i:i + 1],
                    op0=OP.subtract, op1=OP.mult,
                )
                nc.vector.tensor_mul(out=buf[:, i, :], in0=buf[:, i, :], in1=g_bc)
                nc.vector.tensor_add(out=buf[:, i, :], in0=buf[:, i, :], in1=b_bc)

        layernorm_batch(z_sb, lng_bc, lnb_bc)  # z_sb now holds fuse2

        def transpose_tokmajor(buf):
            # [P, NT, D] token-major -> [P, KD, T] feature-major
            bT = xTp.tile([P, KD, T], F32R, tag="xT")
            for i in range(NT):
                for j in range(KD):
                    ps = pst.tile([P, P], F32, tag="ptr")
                    nc.tensor.transpose(ps, buf[:, i, j * P:(j + 1) * P], ident)
                    nc.vector.tensor_copy(out=bT[:, j, i * P:(i + 1) * P], in_=ps)
            return bT

        f2T = transpose_tokmajor(z_sb)

        # h1 = gelu(fuse2 @ W1 + b1) + fuse2  (overwrites z_sb)
        for i in range(NT):
            hp = psb.tile([P, D], F32, tag="pmm")
            for k in range(KD):
                nc.tensor.matmul(
                    hp,
                    _mm(f2T[:, k, i * P:(i + 1) * P], mm_dt),
                    _mm(W1_sb[:, k, :], mm_dt),
                    start=(k == 0), stop=(k == KD - 1),
                )
            nc.vector.tensor_add(out=hp, in0=hp, in1=b1_bc)
            hg = midp.tile([P, D], F32, tag="hg")
            nc.scalar.activation(out=hg, in_=hp, func=AF.Gelu)
            nc.vector.tensor_add(out=z_sb[:, i, :], in0=hg, in1=z_sb[:, i, :])

        layernorm_batch(z_sb, ilng_bc, ilnb_bc)  # z_sb now holds h2

        h2T = transpose_tokmajor(z_sb)

        # h3T = gelu(W2^T @ h2T + b2), then out = h3 @ W3 + b3, per chunk
        TPC = CH // P  # token tiles per chunk (4)
        for c in range(NCH):
            h3T = h3p.tile([P, NM, CH], F32R, tag="h3T")
            for mo in range(NM):
                ps = psb.tile([P, CH], F32, tag="pmm")
                for k in range(KD):
                    nc.tensor.matmul(
                        ps,
                        _mm(W2_sb[:, k, mo * P:(mo + 1) * P], mm_dt),
                        _mm(h2T[:, k, c * CH:(c + 1) * CH], mm_dt),
                        start=(k == 0), stop=(k == KD - 1),
                    )
                nc.scalar.activation(
                    out=h3T[:, mo, :], in_=ps, func=AF.Gelu,
                    bias=b2_sb[:, mo:mo + 1], scale=1.0,
                )
            for it in range(TPC):
                op = psb.tile([P, D], F32, tag="pmm")
                for mo in range(NM):
                    nc.tensor.matmul(
                        op,
                        _mm(h3T[:, mo, it * P:(it + 1) * P], mm_dt),
                        _mm(W3_sb[:, mo, :], mm_dt),
                        start=(mo == 0), stop=(mo == NM - 1),
                    )
                ob = midp.tile([P, D], F32, tag="ob")
                nc.vector.tensor_add(out=ob, in0=op, in1=b3_bc)
                t0 = (c * TPC + it) * P
                nc.sync.dma_start(out_dram[t0:t0 + P, :], ob)

        cctx.close()


_NC_CACHE = {}


def _get_nc():
    key = "full"
    if key not in _NC_CACHE:
        _NC_CACHE[key] = build()
    return _NC_CACHE[key]


WEIGHT_NAMES = (
    "Wl", "bl", "Wr", "br", "Wb", "bb", "Wo", "bo", "ln_g", "ln_b",
    "ir_W1", "ir_b1", "ir_ln_g", "ir_ln_b", "ir_W2", "ir_b2", "ir_W3", "ir_b3",
)


def kernel_with_results(inputs, **spmd_kwargs):
    nc = _get_nc()
    np_in = {k: np.ascontiguousarray(np.asarray(v, dtype=np.float32))
             for k, v in inputs.items()}
    in_maps = []
    for c in range(N_CORES):
        m = {
            "left_embed": np.ascontiguousarray(np_in["left_embed"][c]),
            "right_embed": np.ascontiguousarray(np_in["right_embed"][c]),
            "body_embed": np.ascontiguousarray(np_in["body_embed"][c]),
        }
        for w in WEIGHT_NAMES:
            m[w] = np_in[w]
        in_maps.append(m)
    res = run_bass_kernel_spmd(nc, in_maps, core_ids=list(range(N_CORES)),
                               **spmd_kwargs)
    out = np.stack([res.results[c]["out"] for c in range(N_CORES)], axis=0)
    return out, res


def kernel(**inputs):
    return kernel_with_results(inputs)[0]
